# revision 23
# baseline (speedup 1.0000x reference)
"""Trainium2 Bass kernel for nn_LongRangeDW (dense_cnn).

The module is entirely linear in x:
  s = nnstacking(x)                        (5 shifted copies, clipped to window)
  y = dw1(s) + dw2(s) + dw3(s)             (depthwise 1x1 + 3x3 d8 + 3x3 d12)
  out = pw(y) + x                          (pointwise 5C->C + residual)

Folding the depthwise taps into the pointwise gives, per nnstacking group g
with shift sigma_g and tap tau:
  out[o, p] = sum_{g,t} (W4_g diag(k_{g,t}))[o,:] @ xe[:, p + tau_t + sigma_g]
              + beff[o] + x[o, p]
with xe = zero-extended x: 85 distinct offsets. The folded weight matrices are
quantized to fp8e4 (scaled by 2^7 into e4m3's normal range) and applied two
offsets per instruction with MatmulPerfMode.DoubleRow: lhsT [C, 2, C] holds
two 128x128 blocks, the moving tensor is a 4-dim AP pairing two shifted views
of the fp8 zero-padded SBUF image (pair stride = flat offset delta). Measured
on HW this runs ~1.84x bf16 per folded term.

N_OFF taps of the center group are NOT run as matmuls: their depthwise output
y4 is built by per-channel-scalar tap FMAs on the Vector engine (which is
otherwise idle), followed by one bf16 pointwise matmul term. The first y tile
is emitted in a small 8-row chunk so SB0's pointwise matmul isn't blocked
behind a full 32-row tap batch.

Boundary exactness: composing clipped shifts with zero-padded convs is NOT
the padded composite; the mismatch lives on 8 one-pixel strips (output
rows/cols {7,11,116,120}) reading x's 4 border lines. Those corrections are
linear in x and are folded into xrb on the HOST (exact fp32), so the device
sees no strip machinery at all.

PSUM holds 2^7x the true result: fp8 weights carry 2^7, the residual+bias+
correction tensor xrb enters PSUM through a (2^7 * I) bf16 matmul, and the
idle Scalar engine evacuates with a single Copy-with-scale(2^-7) activation.
The Vector engine runs nothing but tap FMAs; psum slots release through ACT.

Data parallel: batch B=8 -> one image per NeuronCore.
"""

import sys

import numpy as np

sys.path.insert(0, "/opt/trn_rl_repo")

B, C, H, W = 8, 128, 128, 128
PAD = 14            # max |offset| = 13, rounded up
HP = H + 2 * PAD
WP = W + 2 * PAD
N_CORES = 8
SB_ROWS = 8         # output rows per super-block (psum tile = 2 banks)
N_SB = H // SB_ROWS
SUB_ROWS = 4        # rows per matmul (out free dim 512 = one PSUM bank)
SCALE = 128.0       # fp8 weight pre-scale (2^7); descaled at evacuation

SHIFTS = [(1, 0), (-1, 0), (0, 1), (0, -1), (0, 0)]  # nnstacking groups

# Taps of the center group (g=4) are offloaded to the Vector engine as
# per-channel tap FMAs. The offload count ramps across the image: the DVE's
# cumulative y-tile supply curve must stay ahead of the PE's consumption
# curve, and the DVE starts from zero, so early super-blocks offload fewer
# taps (more matmul terms) and late super-blocks offload the whole group.
EARLY_OFF = 7        # g4 taps offloaded for SB 0..RAMP_SB-1
LATE_OFF = 17        # g4 taps offloaded for SB RAMP_SB..15 (all of them)
RAMP_SB = 8


# --------------------------------------------------------------------------
# host-side operator folding
# --------------------------------------------------------------------------

def _group_taps(w1, w2, w3, g):
    """All 17 taps of group g as {(di, dj): kvec[C]} (shift folded in)."""
    sy, sx = SHIFTS[g]
    sl = slice(g * C, (g + 1) * C)
    taps = {}

    def add(di, dj, kv):
        v = taps.setdefault((di, dj), np.zeros(C, np.float64))
        v += kv.astype(np.float64)

    add(sy, sx, w1[sl, 0, 0, 0])
    for w, d in ((w2, 8), (w3, 12)):
        for a in range(3):
            for b in range(3):
                add(sy + (a - 1) * d, sx + (b - 1) * d, w[sl, 0, a, b])
    return taps


def _build_terms(w1, w2, w3, w4):
    """Folds taps into matmul terms, split into the 68 'shared' terms (groups
    0-3, both regions) and the 10 'g4-extra' terms (center-group taps kept as
    matmuls only in the early region).

    Returns dict with:
      sh_offsets/sh_mats: 68 shared terms (paired consecutively)
      g4_offsets/g4_mats: g4 taps[EARLY_OFF:] as matmul terms (early region)
      g4_taps: all 17 sorted g4 tap offsets,  g4_kmat [C, 17]
      w4g4 [C, C]: pointwise block of group 4
    """
    w4m = w4[:, :, 0, 0].astype(np.float64)  # [C, 5C]
    mat_terms = {}
    for g in range(4):
        taps = _group_taps(w1, w2, w3, g)
        for o in sorted(taps):
            M = mat_terms.setdefault(o, np.zeros((C, C), np.float64))
            M += w4m[:, g * C:(g + 1) * C] * taps[o][None, :]
    sh_offsets = sorted(mat_terms)
    sh_mats = np.stack([mat_terms[o] for o in sh_offsets]).astype(np.float32)

    g4 = _group_taps(w1, w2, w3, 4)
    g4_taps = sorted(g4)
    g4_kmat = np.stack([g4[o] for o in g4_taps], axis=1).astype(np.float32)
    w4g4 = w4m[:, 4 * C:5 * C]
    g4_offsets = g4_taps[EARLY_OFF:]
    g4_mats = np.stack([w4g4 * g4[o][None, :] for o in g4_offsets]
                       ).astype(np.float32)
    return dict(sh_offsets=sh_offsets, sh_mats=sh_mats,
                g4_offsets=g4_offsets, g4_mats=g4_mats,
                g4_taps=g4_taps, g4_kmat=g4_kmat,
                w4g4=w4g4.astype(np.float32))


def _build_corrections(w2, w3, w4):
    """24 strip-correction terms (matrices already NEGATED for accumulation).

    Strips j<4: column strips (out col px, read x col src, row shift ty);
    j>=4: row strips. Each strip has 3 taps."""
    w4m = w4[:, :, 0, 0].astype(np.float64)
    strips, mats = [], []
    specs = [
        ("col", 2, 8), ("col", 2, 12), ("col", 3, 12), ("col", 3, 8),
        ("row", 0, 8), ("row", 0, 12), ("row", 1, 12), ("row", 1, 8),
    ]
    for kind, g, d in specs:
        sy, sx = SHIFTS[g]
        sl = slice(g * C, (g + 1) * C)
        w = w2 if d == 8 else w3
        if kind == "col":
            border = -1 if sx == 1 else W
            fixed_out = border - (-d if sx == 1 else d)
            src = border + sx
            shifts = [-d, 0, d]                     # ty values
            tap_b = 0 if sx == 1 else 2
            kvs = [w[sl, 0, a, tap_b] for a in range(3)]
        else:
            border = -1 if sy == 1 else H
            fixed_out = border - (-d if sy == 1 else d)
            src = border + sy
            shifts = [-d, 0, d]                     # tx values
            tap_a = 0 if sy == 1 else 2
            kvs = [w[sl, 0, tap_a, b] for b in range(3)]
        strips.append(dict(kind=kind, fixed_out=fixed_out, src=src, shifts=shifts))
        for kv in kvs:
            mats.append(-(w4m[:, sl] * kv.astype(np.float64)[None, :]))
    return strips, np.stack(mats).astype(np.float32)


def _host_corrections(x, strips, cmats):
    """Per-image boundary corrections, exact fp32: returns [B, C, H, W]."""
    corr = np.zeros((B, C, H, W), np.float32)
    for j, st in enumerate(strips):
        for i, sh in enumerate(st["shifts"]):
            cm = cmats[3 * j + i]                   # [C(out), C(in)], negated
            a, b2 = max(0, -sh), min(H, H - sh)
            if st["kind"] == "col":
                src = x[:, :, a + sh:b2 + sh, st["src"]]        # [B, C, n]
                add = np.einsum("oc,bcn->bon", cm, src)
                corr[:, :, a:b2, st["fixed_out"]] += add
            else:
                src = x[:, :, st["src"], a + sh:b2 + sh]
                add = np.einsum("oc,bcn->bon", cm, src)
                corr[:, :, st["fixed_out"], a:b2] += add
    return corr


def _build_weights(inputs):
    w1, w2, w3, w4 = inputs["w1"], inputs["w2"], inputs["w3"], inputs["w4"]
    b1, b2, b3, b4 = inputs["b1"], inputs["b2"], inputs["b3"], inputs["b4"]
    terms = _build_terms(w1, w2, w3, w4)
    strips, cmats = _build_corrections(w2, w3, w4)

    # fp8 tensor: 68 shared blocks (34 pairs) then 10 g4-early blocks
    # (5 pairs), pair members adjacent in the block dim. Scaled by SCALE.
    mats = np.concatenate([terms["sh_mats"], terms["g4_mats"]], axis=0)
    wt8 = np.ascontiguousarray(
        (mats * SCALE).transpose(2, 0, 1).reshape(C, -1)).astype(np.float32)

    # bf16 tensor: g4 pointwise block (scaled) + SCALE*I for the
    # residual/bias/correction injection.
    bf_blocks = np.stack([terms["w4g4"] * SCALE,
                          np.eye(C, dtype=np.float32) * SCALE])
    wtb = np.ascontiguousarray(
        bf_blocks.transpose(2, 0, 1).reshape(C, -1)).astype(np.float32)

    ks = np.ascontiguousarray(terms["g4_kmat"])  # [C, 17]
    w4m = w4[:, :, 0, 0].astype(np.float64)
    beff = (b4.astype(np.float64)
            + w4m @ (b1 + b2 + b3).astype(np.float64)).astype(np.float32)
    meta = dict(sh_offsets=terms["sh_offsets"],
                g4_offsets=terms["g4_offsets"],
                g4_taps=terms["g4_taps"])
    return wt8, wtb, ks, beff, strips, cmats, meta


# --------------------------------------------------------------------------
# device program
# --------------------------------------------------------------------------

_CACHE = {}

# y-tile schedule: (emit_at_sb, rows_covered); first tile split 8+24 so SB0's
# pointwise matmul is unblocked after a short tap batch.
Y_SCHED = [(0, (0, 8)), (1, (8, 32)), (4, (32, 64)), (8, (64, 96)),
           (12, (96, 128))]


def _build_program(meta):
    import concourse.bacc as bacc
    import concourse.mybir as mybir
    import concourse.tile as tile
    from concourse.ap import AP

    nc = bacc.Bacc("TRN2", target_bir_lowering=False)
    f32 = mybir.dt.float32
    bf16 = mybir.dt.bfloat16
    f8 = mybir.dt.float8e4

    sh_offsets = meta["sh_offsets"]
    g4_offsets = meta["g4_offsets"]
    g4_taps = meta["g4_taps"]
    n_sh = len(sh_offsets)          # 68 shared terms -> 34 pairs
    n_g4 = len(g4_offsets)          # 10 early-only terms -> 5 pairs
    assert n_sh % 2 == 0 and n_g4 % 2 == 0
    n_blk8 = n_sh + n_g4
    PW_BLK, RES_BLK = 0, 1          # bf16 block indices

    xp8_d = nc.dram_tensor("xp8", [C, HP * WP], f8, kind="ExternalInput")
    wt8_d = nc.dram_tensor("wt8", [C, n_blk8 * C], f8, kind="ExternalInput")
    wtb_d = nc.dram_tensor("wtb", [C, 2 * C], bf16, kind="ExternalInput")
    xrb_d = nc.dram_tensor("xrb", [C, H * W], bf16, kind="ExternalInput")
    ks_d = nc.dram_tensor("ks", [C, len(g4_taps)], f32, kind="ExternalInput")
    out_d = nc.dram_tensor("out", [C, H * W], f32, kind="ExternalOutput")

    with tile.TileContext(nc) as tc:
        with (
            tc.tile_pool(name="const", bufs=1) as const,
            tc.tile_pool(name="outp", bufs=3) as outp,
            tc.tile_pool(name="ypA", bufs=1) as ypA,
            tc.tile_pool(name="ypB", bufs=1) as ypB,
            tc.tile_pool(name="ypC", bufs=3) as ypC,
            tc.tile_pool(name="psum", bufs=4, space="PSUM") as psum_pool,
        ):
            xp8_sb = const.tile([C, HP * WP], f8)
            wt8_sb = const.tile([C, n_blk8 * C], f8)
            wtb_sb = const.tile([C, 2 * C], bf16)
            xrb_sb = const.tile([C, H * W], bf16)
            ks_sb = const.tile([C, len(g4_taps)], f32)

            # SWDGE (nc.gpsimd) fans transfers across the SDMA engines;
            # ordered so SB0 (weights, first rows, xrb quarter 0) starts as
            # early as possible.
            nc.sync.dma_start(out=ks_sb, in_=ks_d[:, :])
            ROWS0 = 36
            nc.gpsimd.dma_start(out=xp8_sb[:, :ROWS0 * WP],
                                in_=xp8_d[:, :ROWS0 * WP])
            WT_C0 = 20 * C
            nc.gpsimd.dma_start(out=wt8_sb[:, :WT_C0], in_=wt8_d[:, :WT_C0])
            nc.gpsimd.dma_start(out=wtb_sb, in_=wtb_d[:, :])
            nc.gpsimd.dma_start(out=wt8_sb[:, WT_C0:], in_=wt8_d[:, WT_C0:])
            Q = H * W // 4
            nc.gpsimd.dma_start(out=xrb_sb[:, :Q], in_=xrb_d[:, :Q])
            XP_CHUNK_ROWS = 48
            for r0_ in range(ROWS0, HP, XP_CHUNK_ROWS):
                r1_ = min(r0_ + XP_CHUNK_ROWS, HP)
                nc.gpsimd.dma_start(out=xp8_sb[:, r0_ * WP:r1_ * WP],
                                    in_=xp8_d[:, r0_ * WP:r1_ * WP])
            for q0 in range(Q, H * W, Q):
                nc.gpsimd.dma_start(out=xrb_sb[:, q0:q0 + Q],
                                    in_=xrb_d[:, q0:q0 + Q])

            xp3 = xp8_sb.rearrange("p (r w) -> p r w", w=WP)
            wt83 = wt8_sb.rearrange("p (t c) -> p t c", c=C)
            wtb3 = wtb_sb.rearrange("p (t c) -> p t c", c=C)

            y_pools = {0: ypA, 1: ypB, 2: ypC, 3: ypC, 4: ypC}
            y_sched = {at: (idx, rows) for idx, (at, rows) in enumerate(Y_SCHED)}

            def emit_y(idx, rows, n_taps):
                r_lo, r_hi = rows
                nr = r_hi - r_lo
                pool_key = "C" if idx >= 2 else str(idx)
                y = y_pools[idx].tile([C, nr * W], bf16, name=f"y4_{idx}",
                                      tag=f"y4_{pool_key}")
                for t in range(n_taps):
                    dy, dx = g4_taps[t]
                    xs = xp3[:, PAD + r_lo + dy: PAD + r_lo + dy + nr,
                             PAD + dx: PAD + dx + W]
                    kcol = ks_sb[:, t: t + 1]
                    if t == 0:
                        nc.vector.tensor_scalar_mul(y, xs, kcol)
                    else:
                        nc.vector.scalar_tensor_tensor(
                            y, xs, kcol, y,
                            mybir.AluOpType.mult, mybir.AluOpType.add)
                return (y.rearrange("p (r w) -> p r w", w=W), r_lo)

            def pair_rhs(a0, o1, o2):
                """4-dim AP: two shifted [SUB_ROWS x W] views of xp8, pair
                stride = flat offset delta (DoubleRow moving tensor)."""
                di1, dj1 = o1
                base = xp3[:, a0 + di1: a0 + di1 + SUB_ROWS,
                           PAD + dj1: PAD + dj1 + W]
                delta = (o2[0] - di1) * WP + (o2[1] - dj1)
                dims = [list(d) for d in base.ap]
                nd = [dims[0], [delta, 2]] + dims[1:]
                return AP(base.tensor, base.offset, nd)

            # ---- main loop -------------------------------------------------
            n_sub = SB_ROWS // SUB_ROWS
            cur_y = None
            for s in range(N_SB):
                r0 = s * SB_ROWS
                early = s < RAMP_SB
                if s in y_sched:
                    idx, rows = y_sched[s]
                    n_taps = EARLY_OFF if rows[0] < RAMP_SB * SB_ROWS \
                        else LATE_OFF
                    cur_y = emit_y(idx, rows, n_taps)
                y3, y_base = cur_y
                off_r = r0 - y_base

                # pair list for this region: 34 shared (+5 g4-early)
                pairs = [(wt83[:, 2 * p: 2 * p + 2, :],
                          sh_offsets[2 * p], sh_offsets[2 * p + 1])
                         for p in range(n_sh // 2)]
                if early:
                    pairs += [(wt83[:, n_sh + 2 * p: n_sh + 2 * p + 2, :],
                               g4_offsets[2 * p], g4_offsets[2 * p + 1])
                              for p in range(n_g4 // 2)]

                psum = psum_pool.tile([C, SB_ROWS * W], f32, tag="acc")
                for p, (lhsT, o1, o2) in enumerate(pairs):
                    for u in range(n_sub):
                        a0 = PAD + r0 + u * SUB_ROWS
                        nc.tensor.matmul(
                            psum[:, u * SUB_ROWS * W:(u + 1) * SUB_ROWS * W],
                            lhsT, pair_rhs(a0, o1, o2),
                            start=(p == 0), stop=False,
                            perf_mode=mybir.MatmulPerfMode.DoubleRow)
                for u in range(n_sub):
                    lo = off_r + u * SUB_ROWS
                    nc.tensor.matmul(
                        psum[:, u * SUB_ROWS * W:(u + 1) * SUB_ROWS * W],
                        wtb3[:, PW_BLK, :],
                        y3[:, lo: lo + SUB_ROWS, :],
                        start=False, stop=False)
                # residual + bias + boundary corrections, pre-combined in xrb
                for u in range(n_sub):
                    q0 = (r0 + u * SUB_ROWS) * W
                    nc.tensor.matmul(
                        psum[:, u * SUB_ROWS * W:(u + 1) * SUB_ROWS * W],
                        wtb3[:, RES_BLK, :],
                        xrb_sb[:, q0: q0 + SUB_ROWS * W],
                        start=False, stop=(u == n_sub - 1))

                # idle Scalar engine evacuates: out = psum * 2^-7
                out_sb = outp.tile([C, SB_ROWS * W], f32)
                nc.scalar.activation(out_sb, psum,
                                     mybir.ActivationFunctionType.Copy,
                                     bias=0.0, scale=1.0 / SCALE)
                nc.gpsimd.dma_start(out=out_d[:, r0 * W:(r0 + SB_ROWS) * W],
                                    in_=out_sb)
    nc.finalize()
    return nc


def _make_in_maps(inputs):
    x = np.ascontiguousarray(inputs["x"], dtype=np.float32)
    wt8, wtb, ks, beff, strips, cmats, meta = _build_weights(inputs)
    if "nc" not in _CACHE:
        _CACHE["nc"] = _build_program(meta)

    import ml_dtypes
    bf = ml_dtypes.bfloat16
    f8 = ml_dtypes.float8_e4m3
    xpad = np.zeros((B, C, HP, WP), f8)
    xpad[:, :, PAD:PAD + H, PAD:PAD + W] = x.astype(f8)
    corr = _host_corrections(x, strips, cmats)
    xrb = (x + beff[None, :, None, None] + corr).astype(bf)
    wt8_q = wt8.astype(f8)
    wtb_q = wtb.astype(bf)
    ksc = np.ascontiguousarray(ks)
    return [
        {
            "xp8": np.ascontiguousarray(xpad[b].reshape(C, HP * WP)),
            "wt8": wt8_q,
            "wtb": wtb_q,
            "xrb": np.ascontiguousarray(xrb[b].reshape(C, H * W)),
            "ks": ksc,
        }
        for b in range(B)
    ]


def kernel(**inputs):
    in_maps = _make_in_maps(inputs)
    from concourse.bass_utils import run_bass_kernel_spmd
    res = run_bass_kernel_spmd(_CACHE["nc"], in_maps, core_ids=list(range(N_CORES)))
    out = np.stack([res.results[b]["out"].reshape(C, H, W) for b in range(B)])
    return out.astype(np.float32)


# revision 24
# speedup vs baseline: 1.0001x; 1.0001x over previous
"""Trainium2 Bass kernel for nn_LongRangeDW (dense_cnn).

The module is entirely linear in x:
  s = nnstacking(x)                        (5 shifted copies, clipped to window)
  y = dw1(s) + dw2(s) + dw3(s)             (depthwise 1x1 + 3x3 d8 + 3x3 d12)
  out = pw(y) + x                          (pointwise 5C->C + residual)

Folding the depthwise taps into the pointwise gives, per nnstacking group g
with shift sigma_g and tap tau:
  out[o, p] = sum_{g,t} (W4_g diag(k_{g,t}))[o,:] @ xe[:, p + tau_t + sigma_g]
              + beff[o] + x[o, p]
with xe = zero-extended x: 85 distinct offsets. The folded weight matrices are
quantized to fp8e4 (scaled by 2^7 into e4m3's normal range) and applied two
offsets per instruction with MatmulPerfMode.DoubleRow: lhsT [C, 2, C] holds
two 128x128 blocks, the moving tensor is a 4-dim AP pairing two shifted views
of the fp8 zero-padded SBUF image (pair stride = flat offset delta). Measured
on HW this runs ~1.84x bf16 per folded term.

N_OFF taps of the center group are NOT run as matmuls: their depthwise output
y4 is built by per-channel-scalar tap FMAs on the Vector engine (which is
otherwise idle), followed by one bf16 pointwise matmul term. The first y tile
is emitted in a small 8-row chunk so SB0's pointwise matmul isn't blocked
behind a full 32-row tap batch.

Boundary exactness: composing clipped shifts with zero-padded convs is NOT
the padded composite; the mismatch lives on 8 one-pixel strips (output
rows/cols {7,11,116,120}) reading x's 4 border lines. Those corrections are
linear in x and are folded into xrb on the HOST (exact fp32), so the device
sees no strip machinery at all.

PSUM holds 2^7x the true result: fp8 weights carry 2^7, the residual+bias+
correction tensor xrb enters PSUM through a (2^7 * I) bf16 matmul, and the
idle Scalar engine evacuates with a single Copy-with-scale(2^-7) activation.
The Vector engine runs nothing but tap FMAs; psum slots release through ACT.

Data parallel: batch B=8 -> one image per NeuronCore.
"""

import sys

import numpy as np

sys.path.insert(0, "/opt/trn_rl_repo")

B, C, H, W = 8, 128, 128, 128
PAD = 14            # max |offset| = 13, rounded up
HP = H + 2 * PAD
WP = W + 2 * PAD
N_CORES = 8
SB_ROWS = 8         # output rows per super-block (psum tile = 2 banks)
N_SB = H // SB_ROWS
SUB_ROWS = 4        # rows per matmul (out free dim 512 = one PSUM bank)
SCALE = 128.0       # fp8 weight pre-scale (2^7); descaled at evacuation

SHIFTS = [(1, 0), (-1, 0), (0, 1), (0, -1), (0, 0)]  # nnstacking groups

# Taps of the center group (g=4) are offloaded to the Vector engine as
# per-channel tap FMAs. The offload count ramps across the image: the DVE's
# cumulative y-tile supply curve must stay ahead of the PE's consumption
# curve, and the DVE starts from zero, so early super-blocks offload fewer
# taps (more matmul terms) and late super-blocks offload the whole group.
EARLY_OFF = 7        # g4 taps offloaded for SB 0..RAMP_SB-1
LATE_OFF = 17        # g4 taps offloaded for SB RAMP_SB..15 (all of them)
RAMP_SB = 8


# --------------------------------------------------------------------------
# host-side operator folding
# --------------------------------------------------------------------------

def _group_taps(w1, w2, w3, g):
    """All 17 taps of group g as {(di, dj): kvec[C]} (shift folded in)."""
    sy, sx = SHIFTS[g]
    sl = slice(g * C, (g + 1) * C)
    taps = {}

    def add(di, dj, kv):
        v = taps.setdefault((di, dj), np.zeros(C, np.float64))
        v += kv.astype(np.float64)

    add(sy, sx, w1[sl, 0, 0, 0])
    for w, d in ((w2, 8), (w3, 12)):
        for a in range(3):
            for b in range(3):
                add(sy + (a - 1) * d, sx + (b - 1) * d, w[sl, 0, a, b])
    return taps


def _build_terms(w1, w2, w3, w4):
    """Folds taps into matmul terms, split into the 68 'shared' terms (groups
    0-3, both regions) and the 10 'g4-extra' terms (center-group taps kept as
    matmuls only in the early region).

    Returns dict with:
      sh_offsets/sh_mats: 68 shared terms (paired consecutively)
      g4_offsets/g4_mats: g4 taps[EARLY_OFF:] as matmul terms (early region)
      g4_taps: all 17 sorted g4 tap offsets,  g4_kmat [C, 17]
      w4g4 [C, C]: pointwise block of group 4
    """
    w4m = w4[:, :, 0, 0].astype(np.float64)  # [C, 5C]
    mat_terms = {}
    for g in range(4):
        taps = _group_taps(w1, w2, w3, g)
        for o in sorted(taps):
            M = mat_terms.setdefault(o, np.zeros((C, C), np.float64))
            M += w4m[:, g * C:(g + 1) * C] * taps[o][None, :]
    sh_offsets = sorted(mat_terms)
    sh_mats = np.stack([mat_terms[o] for o in sh_offsets]).astype(np.float32)

    g4 = _group_taps(w1, w2, w3, 4)
    g4_taps = sorted(g4)
    g4_kmat = np.stack([g4[o] for o in g4_taps], axis=1).astype(np.float32)
    w4g4 = w4m[:, 4 * C:5 * C]
    g4_offsets = g4_taps[EARLY_OFF:]
    g4_mats = np.stack([w4g4 * g4[o][None, :] for o in g4_offsets]
                       ).astype(np.float32)
    return dict(sh_offsets=sh_offsets, sh_mats=sh_mats,
                g4_offsets=g4_offsets, g4_mats=g4_mats,
                g4_taps=g4_taps, g4_kmat=g4_kmat,
                w4g4=w4g4.astype(np.float32))


def _build_corrections(w2, w3, w4):
    """24 strip-correction terms (matrices already NEGATED for accumulation).

    Strips j<4: column strips (out col px, read x col src, row shift ty);
    j>=4: row strips. Each strip has 3 taps."""
    w4m = w4[:, :, 0, 0].astype(np.float64)
    strips, mats = [], []
    specs = [
        ("col", 2, 8), ("col", 2, 12), ("col", 3, 12), ("col", 3, 8),
        ("row", 0, 8), ("row", 0, 12), ("row", 1, 12), ("row", 1, 8),
    ]
    for kind, g, d in specs:
        sy, sx = SHIFTS[g]
        sl = slice(g * C, (g + 1) * C)
        w = w2 if d == 8 else w3
        if kind == "col":
            border = -1 if sx == 1 else W
            fixed_out = border - (-d if sx == 1 else d)
            src = border + sx
            shifts = [-d, 0, d]                     # ty values
            tap_b = 0 if sx == 1 else 2
            kvs = [w[sl, 0, a, tap_b] for a in range(3)]
        else:
            border = -1 if sy == 1 else H
            fixed_out = border - (-d if sy == 1 else d)
            src = border + sy
            shifts = [-d, 0, d]                     # tx values
            tap_a = 0 if sy == 1 else 2
            kvs = [w[sl, 0, tap_a, b] for b in range(3)]
        strips.append(dict(kind=kind, fixed_out=fixed_out, src=src, shifts=shifts))
        for kv in kvs:
            mats.append(-(w4m[:, sl] * kv.astype(np.float64)[None, :]))
    return strips, np.stack(mats).astype(np.float32)


def _host_corrections(x, strips, cmats):
    """Per-image boundary corrections, exact fp32: returns [B, C, H, W]."""
    corr = np.zeros((B, C, H, W), np.float32)
    for j, st in enumerate(strips):
        for i, sh in enumerate(st["shifts"]):
            cm = cmats[3 * j + i]                   # [C(out), C(in)], negated
            a, b2 = max(0, -sh), min(H, H - sh)
            if st["kind"] == "col":
                src = x[:, :, a + sh:b2 + sh, st["src"]]        # [B, C, n]
                add = np.einsum("oc,bcn->bon", cm, src)
                corr[:, :, a:b2, st["fixed_out"]] += add
            else:
                src = x[:, :, st["src"], a + sh:b2 + sh]
                add = np.einsum("oc,bcn->bon", cm, src)
                corr[:, :, st["fixed_out"], a:b2] += add
    return corr


def _build_weights(inputs):
    w1, w2, w3, w4 = inputs["w1"], inputs["w2"], inputs["w3"], inputs["w4"]
    b1, b2, b3, b4 = inputs["b1"], inputs["b2"], inputs["b3"], inputs["b4"]
    terms = _build_terms(w1, w2, w3, w4)
    strips, cmats = _build_corrections(w2, w3, w4)

    # fp8 tensor: 68 shared blocks (34 pairs) then 10 g4-early blocks
    # (5 pairs), pair members adjacent in the block dim. Scaled by SCALE.
    mats = np.concatenate([terms["sh_mats"], terms["g4_mats"]], axis=0)
    wt8 = np.ascontiguousarray(
        (mats * SCALE).transpose(2, 0, 1).reshape(C, -1)).astype(np.float32)

    # bf16 tensor: g4 pointwise block (scaled) + SCALE*I for the
    # residual/bias/correction injection.
    bf_blocks = np.stack([terms["w4g4"] * SCALE,
                          np.eye(C, dtype=np.float32) * SCALE])
    wtb = np.ascontiguousarray(
        bf_blocks.transpose(2, 0, 1).reshape(C, -1)).astype(np.float32)

    ks = np.ascontiguousarray(terms["g4_kmat"])  # [C, 17]
    w4m = w4[:, :, 0, 0].astype(np.float64)
    beff = (b4.astype(np.float64)
            + w4m @ (b1 + b2 + b3).astype(np.float64)).astype(np.float32)
    meta = dict(sh_offsets=terms["sh_offsets"],
                g4_offsets=terms["g4_offsets"],
                g4_taps=terms["g4_taps"])
    return wt8, wtb, ks, beff, strips, cmats, meta


# --------------------------------------------------------------------------
# device program
# --------------------------------------------------------------------------

_CACHE = {}

# y-tile schedule: (emit_at_sb, rows_covered); first tile split 8+24 so SB0's
# pointwise matmul is unblocked after a short tap batch.
Y_SCHED = [(0, (0, 8)), (1, (8, 32)), (4, (32, 64)), (8, (64, 96)),
           (12, (96, 128))]


def _build_program(meta):
    import concourse.bacc as bacc
    import concourse.mybir as mybir
    import concourse.tile as tile
    from concourse.ap import AP

    nc = bacc.Bacc("TRN2", target_bir_lowering=False)
    f32 = mybir.dt.float32
    bf16 = mybir.dt.bfloat16
    f8 = mybir.dt.float8e4

    sh_offsets = meta["sh_offsets"]
    g4_offsets = meta["g4_offsets"]
    g4_taps = meta["g4_taps"]
    n_sh = len(sh_offsets)          # 68 shared terms -> 34 pairs
    n_g4 = len(g4_offsets)          # 10 early-only terms -> 5 pairs
    assert n_sh % 2 == 0 and n_g4 % 2 == 0
    n_blk8 = n_sh + n_g4
    PW_BLK, RES_BLK = 0, 1          # bf16 block indices

    xp8_d = nc.dram_tensor("xp8", [C, HP * WP], f8, kind="ExternalInput")
    wt8_d = nc.dram_tensor("wt8", [C, n_blk8 * C], f8, kind="ExternalInput")
    wtb_d = nc.dram_tensor("wtb", [C, 2 * C], bf16, kind="ExternalInput")
    xrb_d = nc.dram_tensor("xrb", [C, H * W], bf16, kind="ExternalInput")
    ks_d = nc.dram_tensor("ks", [C, len(g4_taps)], f32, kind="ExternalInput")
    out_d = nc.dram_tensor("out", [C, H * W], f32, kind="ExternalOutput")

    with tile.TileContext(nc) as tc:
        with (
            tc.tile_pool(name="const", bufs=1) as const,
            tc.tile_pool(name="outp", bufs=3) as outp,
            tc.tile_pool(name="ypA", bufs=1) as ypA,
            tc.tile_pool(name="ypB", bufs=1) as ypB,
            tc.tile_pool(name="ypC", bufs=3) as ypC,
            tc.tile_pool(name="psum", bufs=4, space="PSUM") as psum_pool,
        ):
            xp8_sb = const.tile([C, HP * WP], f8)
            wt8_sb = const.tile([C, n_blk8 * C], f8)
            wtb_sb = const.tile([C, 2 * C], bf16)
            xrb_sb = const.tile([C, H * W], bf16)
            ks_sb = const.tile([C, len(g4_taps)], f32)

            # SWDGE (nc.gpsimd) fans transfers across the SDMA engines;
            # ordered so SB0 (weights, first rows, xrb quarter 0) starts as
            # early as possible.
            nc.sync.dma_start(out=ks_sb, in_=ks_d[:, :])
            ROWS0 = 36
            nc.gpsimd.dma_start(out=xp8_sb[:, :ROWS0 * WP],
                                in_=xp8_d[:, :ROWS0 * WP])
            WT_C0 = 20 * C
            nc.gpsimd.dma_start(out=wt8_sb[:, :WT_C0], in_=wt8_d[:, :WT_C0])
            nc.gpsimd.dma_start(out=wtb_sb, in_=wtb_d[:, :])
            nc.gpsimd.dma_start(out=wt8_sb[:, WT_C0:], in_=wt8_d[:, WT_C0:])
            Q = H * W // 4
            nc.gpsimd.dma_start(out=xrb_sb[:, :Q], in_=xrb_d[:, :Q])
            XP_CHUNK_ROWS = 48
            for r0_ in range(ROWS0, HP, XP_CHUNK_ROWS):
                r1_ = min(r0_ + XP_CHUNK_ROWS, HP)
                nc.gpsimd.dma_start(out=xp8_sb[:, r0_ * WP:r1_ * WP],
                                    in_=xp8_d[:, r0_ * WP:r1_ * WP])
            for q0 in range(Q, H * W, Q):
                nc.gpsimd.dma_start(out=xrb_sb[:, q0:q0 + Q],
                                    in_=xrb_d[:, q0:q0 + Q])

            xp3 = xp8_sb.rearrange("p (r w) -> p r w", w=WP)
            wt83 = wt8_sb.rearrange("p (t c) -> p t c", c=C)
            wtb3 = wtb_sb.rearrange("p (t c) -> p t c", c=C)

            # PE p-state warmup: the tensor engine clock ramps to full speed
            # only after ~3us of continuous execution. Run dummy matmuls on a
            # zeroed scratch tile during the startup-DMA window so the real
            # stream starts at 2.4 GHz. The dummy psum group occupies pool
            # slot 0, which SB3 reuses long after the warmup retires.
            scratch = const.tile([C, 640], f8)
            nc.scalar.memzero(scratch)
            dpsum = psum_pool.tile([C, SB_ROWS * W], f32, tag="acc")
            N_WARM = 40
            for i in range(N_WARM):
                nc.tensor.matmul(dpsum[:, :512], scratch[:, :C],
                                 scratch[:, C:C + 512],
                                 start=(i == 0), stop=(i == N_WARM - 1))

            y_pools = {0: ypA, 1: ypB, 2: ypC, 3: ypC, 4: ypC}
            y_sched = {at: (idx, rows) for idx, (at, rows) in enumerate(Y_SCHED)}

            def emit_y(idx, rows, n_taps):
                r_lo, r_hi = rows
                nr = r_hi - r_lo
                pool_key = "C" if idx >= 2 else str(idx)
                y = y_pools[idx].tile([C, nr * W], bf16, name=f"y4_{idx}",
                                      tag=f"y4_{pool_key}")
                for t in range(n_taps):
                    dy, dx = g4_taps[t]
                    xs = xp3[:, PAD + r_lo + dy: PAD + r_lo + dy + nr,
                             PAD + dx: PAD + dx + W]
                    kcol = ks_sb[:, t: t + 1]
                    if t == 0:
                        nc.vector.tensor_scalar_mul(y, xs, kcol)
                    else:
                        nc.vector.scalar_tensor_tensor(
                            y, xs, kcol, y,
                            mybir.AluOpType.mult, mybir.AluOpType.add)
                return (y.rearrange("p (r w) -> p r w", w=W), r_lo)

            def pair_rhs(a0, o1, o2):
                """4-dim AP: two shifted [SUB_ROWS x W] views of xp8, pair
                stride = flat offset delta (DoubleRow moving tensor)."""
                di1, dj1 = o1
                base = xp3[:, a0 + di1: a0 + di1 + SUB_ROWS,
                           PAD + dj1: PAD + dj1 + W]
                delta = (o2[0] - di1) * WP + (o2[1] - dj1)
                dims = [list(d) for d in base.ap]
                nd = [dims[0], [delta, 2]] + dims[1:]
                return AP(base.tensor, base.offset, nd)

            # ---- main loop -------------------------------------------------
            n_sub = SB_ROWS // SUB_ROWS
            cur_y = None
            for s in range(N_SB):
                r0 = s * SB_ROWS
                early = s < RAMP_SB
                if s in y_sched:
                    idx, rows = y_sched[s]
                    n_taps = EARLY_OFF if rows[0] < RAMP_SB * SB_ROWS \
                        else LATE_OFF
                    cur_y = emit_y(idx, rows, n_taps)
                y3, y_base = cur_y
                off_r = r0 - y_base

                # pair list for this region: 34 shared (+5 g4-early)
                pairs = [(wt83[:, 2 * p: 2 * p + 2, :],
                          sh_offsets[2 * p], sh_offsets[2 * p + 1])
                         for p in range(n_sh // 2)]
                if early:
                    pairs += [(wt83[:, n_sh + 2 * p: n_sh + 2 * p + 2, :],
                               g4_offsets[2 * p], g4_offsets[2 * p + 1])
                              for p in range(n_g4 // 2)]

                psum = psum_pool.tile([C, SB_ROWS * W], f32, tag="acc")
                for p, (lhsT, o1, o2) in enumerate(pairs):
                    for u in range(n_sub):
                        a0 = PAD + r0 + u * SUB_ROWS
                        nc.tensor.matmul(
                            psum[:, u * SUB_ROWS * W:(u + 1) * SUB_ROWS * W],
                            lhsT, pair_rhs(a0, o1, o2),
                            start=(p == 0), stop=False,
                            perf_mode=mybir.MatmulPerfMode.DoubleRow)
                for u in range(n_sub):
                    lo = off_r + u * SUB_ROWS
                    nc.tensor.matmul(
                        psum[:, u * SUB_ROWS * W:(u + 1) * SUB_ROWS * W],
                        wtb3[:, PW_BLK, :],
                        y3[:, lo: lo + SUB_ROWS, :],
                        start=False, stop=False)
                # residual + bias + boundary corrections, pre-combined in xrb
                for u in range(n_sub):
                    q0 = (r0 + u * SUB_ROWS) * W
                    nc.tensor.matmul(
                        psum[:, u * SUB_ROWS * W:(u + 1) * SUB_ROWS * W],
                        wtb3[:, RES_BLK, :],
                        xrb_sb[:, q0: q0 + SUB_ROWS * W],
                        start=False, stop=(u == n_sub - 1))

                # idle Scalar engine evacuates: out = psum * 2^-7
                out_sb = outp.tile([C, SB_ROWS * W], f32)
                nc.scalar.activation(out_sb, psum,
                                     mybir.ActivationFunctionType.Copy,
                                     bias=0.0, scale=1.0 / SCALE)
                nc.gpsimd.dma_start(out=out_d[:, r0 * W:(r0 + SB_ROWS) * W],
                                    in_=out_sb)
    nc.finalize()
    return nc


def _make_in_maps(inputs):
    x = np.ascontiguousarray(inputs["x"], dtype=np.float32)
    wt8, wtb, ks, beff, strips, cmats, meta = _build_weights(inputs)
    if "nc" not in _CACHE:
        _CACHE["nc"] = _build_program(meta)

    import ml_dtypes
    bf = ml_dtypes.bfloat16
    f8 = ml_dtypes.float8_e4m3
    xpad = np.zeros((B, C, HP, WP), f8)
    xpad[:, :, PAD:PAD + H, PAD:PAD + W] = x.astype(f8)
    corr = _host_corrections(x, strips, cmats)
    xrb = (x + beff[None, :, None, None] + corr).astype(bf)
    wt8_q = wt8.astype(f8)
    wtb_q = wtb.astype(bf)
    ksc = np.ascontiguousarray(ks)
    return [
        {
            "xp8": np.ascontiguousarray(xpad[b].reshape(C, HP * WP)),
            "wt8": wt8_q,
            "wtb": wtb_q,
            "xrb": np.ascontiguousarray(xrb[b].reshape(C, H * W)),
            "ks": ksc,
        }
        for b in range(B)
    ]


def kernel(**inputs):
    in_maps = _make_in_maps(inputs)
    from concourse.bass_utils import run_bass_kernel_spmd
    res = run_bass_kernel_spmd(_CACHE["nc"], in_maps, core_ids=list(range(N_CORES)))
    out = np.stack([res.results[b]["out"].reshape(C, H, W) for b in range(B)])
    return out.astype(np.float32)


# revision 25
# speedup vs baseline: 1.0063x; 1.0062x over previous
"""Trainium2 Bass kernel for nn_LongRangeDW (dense_cnn).

The module is entirely linear in x:
  s = nnstacking(x)                        (5 shifted copies, clipped to window)
  y = dw1(s) + dw2(s) + dw3(s)             (depthwise 1x1 + 3x3 d8 + 3x3 d12)
  out = pw(y) + x                          (pointwise 5C->C + residual)

Folding the depthwise taps into the pointwise gives, per nnstacking group g
with shift sigma_g and tap tau:
  out[o, p] = sum_{g,t} (W4_g diag(k_{g,t}))[o,:] @ xe[:, p + tau_t + sigma_g]
              + beff[o] + x[o, p]
with xe = zero-extended x: 85 distinct offsets. The folded weight matrices are
quantized to fp8e4 (scaled by 2^7 into e4m3's normal range) and applied two
offsets per instruction with MatmulPerfMode.DoubleRow: lhsT [C, 2, C] holds
two 128x128 blocks, the moving tensor is a 4-dim AP pairing two shifted views
of the fp8 zero-padded SBUF image (pair stride = flat offset delta). Measured
on HW this runs ~1.84x bf16 per folded term.

N_OFF taps of the center group are NOT run as matmuls: their depthwise output
y4 is built by per-channel-scalar tap FMAs on the Vector engine (which is
otherwise idle), followed by one bf16 pointwise matmul term. The first y tile
is emitted in a small 8-row chunk so SB0's pointwise matmul isn't blocked
behind a full 32-row tap batch.

Boundary exactness: composing clipped shifts with zero-padded convs is NOT
the padded composite; the mismatch lives on 8 one-pixel strips (output
rows/cols {7,11,116,120}) reading x's 4 border lines. Those corrections are
linear in x and are folded into xrb on the HOST (exact fp32), so the device
sees no strip machinery at all.

PSUM holds 2^7x the true result: fp8 weights carry 2^7, the residual+bias+
correction tensor xrb enters PSUM through a (2^7 * I) bf16 matmul, and the
idle Scalar engine evacuates with a single Copy-with-scale(2^-7) activation.
The Vector engine runs nothing but tap FMAs; psum slots release through ACT.

Data parallel: batch B=8 -> one image per NeuronCore.
"""

import sys

import numpy as np

sys.path.insert(0, "/opt/trn_rl_repo")

B, C, H, W = 8, 128, 128, 128
PAD = 14            # max |offset| = 13, rounded up
HP = H + 2 * PAD
WP = W + 2 * PAD
N_CORES = 8
SB_ROWS = 8         # output rows per super-block (psum tile = 2 banks)
N_SB = H // SB_ROWS
SUB_ROWS = 4        # rows per matmul (out free dim 512 = one PSUM bank)
SCALE = 128.0       # fp8 weight pre-scale (2^7); descaled at evacuation

SHIFTS = [(1, 0), (-1, 0), (0, 1), (0, -1), (0, 0)]  # nnstacking groups

# Taps of the center group (g=4) are offloaded to the Vector engine as
# per-channel tap FMAs. The offload count ramps across the image: the DVE's
# cumulative y-tile supply curve must stay ahead of the PE's consumption
# curve, and the DVE starts from zero, so early super-blocks offload fewer
# taps (more matmul terms) and late super-blocks offload the whole group.
EARLY_OFF = 7        # g4 taps offloaded for SB 0..RAMP_SB-1
LATE_OFF = 17        # g4 taps offloaded for SB RAMP_SB..15 (all of them)
RAMP_SB = 8


# --------------------------------------------------------------------------
# host-side operator folding
# --------------------------------------------------------------------------

def _group_taps(w1, w2, w3, g):
    """All 17 taps of group g as {(di, dj): kvec[C]} (shift folded in)."""
    sy, sx = SHIFTS[g]
    sl = slice(g * C, (g + 1) * C)
    taps = {}

    def add(di, dj, kv):
        v = taps.setdefault((di, dj), np.zeros(C, np.float64))
        v += kv.astype(np.float64)

    add(sy, sx, w1[sl, 0, 0, 0])
    for w, d in ((w2, 8), (w3, 12)):
        for a in range(3):
            for b in range(3):
                add(sy + (a - 1) * d, sx + (b - 1) * d, w[sl, 0, a, b])
    return taps


def _build_terms(w1, w2, w3, w4):
    """Folds taps into matmul terms, split into the 68 'shared' terms (groups
    0-3, both regions) and the 10 'g4-extra' terms (center-group taps kept as
    matmuls only in the early region).

    Returns dict with:
      sh_offsets/sh_mats: 68 shared terms (paired consecutively)
      g4_offsets/g4_mats: g4 taps[EARLY_OFF:] as matmul terms (early region)
      g4_taps: all 17 sorted g4 tap offsets,  g4_kmat [C, 17]
      w4g4 [C, C]: pointwise block of group 4
    """
    w4m = w4[:, :, 0, 0].astype(np.float64)  # [C, 5C]
    mat_terms = {}
    for g in range(4):
        taps = _group_taps(w1, w2, w3, g)
        for o in sorted(taps):
            M = mat_terms.setdefault(o, np.zeros((C, C), np.float64))
            M += w4m[:, g * C:(g + 1) * C] * taps[o][None, :]
    sh_offsets = sorted(mat_terms)
    sh_mats = np.stack([mat_terms[o] for o in sh_offsets]).astype(np.float32)

    g4 = _group_taps(w1, w2, w3, 4)
    g4_taps = sorted(g4)
    g4_kmat = np.stack([g4[o] for o in g4_taps], axis=1).astype(np.float32)
    w4g4 = w4m[:, 4 * C:5 * C]
    g4_offsets = g4_taps[EARLY_OFF:]
    g4_mats = np.stack([w4g4 * g4[o][None, :] for o in g4_offsets]
                       ).astype(np.float32)
    return dict(sh_offsets=sh_offsets, sh_mats=sh_mats,
                g4_offsets=g4_offsets, g4_mats=g4_mats,
                g4_taps=g4_taps, g4_kmat=g4_kmat,
                w4g4=w4g4.astype(np.float32))


def _build_corrections(w2, w3, w4):
    """24 strip-correction terms (matrices already NEGATED for accumulation).

    Strips j<4: column strips (out col px, read x col src, row shift ty);
    j>=4: row strips. Each strip has 3 taps."""
    w4m = w4[:, :, 0, 0].astype(np.float64)
    strips, mats = [], []
    specs = [
        ("col", 2, 8), ("col", 2, 12), ("col", 3, 12), ("col", 3, 8),
        ("row", 0, 8), ("row", 0, 12), ("row", 1, 12), ("row", 1, 8),
    ]
    for kind, g, d in specs:
        sy, sx = SHIFTS[g]
        sl = slice(g * C, (g + 1) * C)
        w = w2 if d == 8 else w3
        if kind == "col":
            border = -1 if sx == 1 else W
            fixed_out = border - (-d if sx == 1 else d)
            src = border + sx
            shifts = [-d, 0, d]                     # ty values
            tap_b = 0 if sx == 1 else 2
            kvs = [w[sl, 0, a, tap_b] for a in range(3)]
        else:
            border = -1 if sy == 1 else H
            fixed_out = border - (-d if sy == 1 else d)
            src = border + sy
            shifts = [-d, 0, d]                     # tx values
            tap_a = 0 if sy == 1 else 2
            kvs = [w[sl, 0, tap_a, b] for b in range(3)]
        strips.append(dict(kind=kind, fixed_out=fixed_out, src=src, shifts=shifts))
        for kv in kvs:
            mats.append(-(w4m[:, sl] * kv.astype(np.float64)[None, :]))
    return strips, np.stack(mats).astype(np.float32)


def _host_corrections(x, strips, cmats):
    """Per-image boundary corrections, exact fp32: returns [B, C, H, W]."""
    corr = np.zeros((B, C, H, W), np.float32)
    for j, st in enumerate(strips):
        for i, sh in enumerate(st["shifts"]):
            cm = cmats[3 * j + i]                   # [C(out), C(in)], negated
            a, b2 = max(0, -sh), min(H, H - sh)
            if st["kind"] == "col":
                src = x[:, :, a + sh:b2 + sh, st["src"]]        # [B, C, n]
                add = np.einsum("oc,bcn->bon", cm, src)
                corr[:, :, a:b2, st["fixed_out"]] += add
            else:
                src = x[:, :, st["src"], a + sh:b2 + sh]
                add = np.einsum("oc,bcn->bon", cm, src)
                corr[:, :, st["fixed_out"], a:b2] += add
    return corr


def _build_weights(inputs):
    w1, w2, w3, w4 = inputs["w1"], inputs["w2"], inputs["w3"], inputs["w4"]
    b1, b2, b3, b4 = inputs["b1"], inputs["b2"], inputs["b3"], inputs["b4"]
    terms = _build_terms(w1, w2, w3, w4)
    strips, cmats = _build_corrections(w2, w3, w4)

    # fp8 tensor: 68 shared blocks (34 pairs) then 10 g4-early blocks
    # (5 pairs), pair members adjacent in the block dim. Scaled by SCALE.
    mats = np.concatenate([terms["sh_mats"], terms["g4_mats"]], axis=0)
    wt8 = np.ascontiguousarray(
        (mats * SCALE).transpose(2, 0, 1).reshape(C, -1)).astype(np.float32)

    # bf16 tensor: g4 pointwise block (scaled) + SCALE*I for the
    # residual/bias/correction injection.
    bf_blocks = np.stack([terms["w4g4"] * SCALE,
                          np.eye(C, dtype=np.float32) * SCALE])
    wtb = np.ascontiguousarray(
        bf_blocks.transpose(2, 0, 1).reshape(C, -1)).astype(np.float32)

    ks = np.ascontiguousarray(terms["g4_kmat"])  # [C, 17]
    w4m = w4[:, :, 0, 0].astype(np.float64)
    beff = (b4.astype(np.float64)
            + w4m @ (b1 + b2 + b3).astype(np.float64)).astype(np.float32)
    meta = dict(sh_offsets=terms["sh_offsets"],
                g4_offsets=terms["g4_offsets"],
                g4_taps=terms["g4_taps"])
    return wt8, wtb, ks, beff, strips, cmats, meta


# --------------------------------------------------------------------------
# device program
# --------------------------------------------------------------------------

_CACHE = {}

# y-tile schedule: (emit_at_sb, rows_covered); first tile split 8+24 so SB0's
# pointwise matmul is unblocked after a short tap batch.
Y_SCHED = [(0, (0, 8)), (1, (8, 32)), (4, (32, 64)), (8, (64, 96)),
           (12, (96, 128))]


def _build_program(meta):
    import concourse.bacc as bacc
    import concourse.mybir as mybir
    import concourse.tile as tile
    from concourse.ap import AP

    nc = bacc.Bacc("TRN2", target_bir_lowering=False)
    f32 = mybir.dt.float32
    bf16 = mybir.dt.bfloat16
    f8 = mybir.dt.float8e4

    sh_offsets = meta["sh_offsets"]
    g4_offsets = meta["g4_offsets"]
    g4_taps = meta["g4_taps"]
    n_sh = len(sh_offsets)          # 68 shared terms -> 34 pairs
    n_g4 = len(g4_offsets)          # 10 early-only terms -> 5 pairs
    assert n_sh % 2 == 0 and n_g4 % 2 == 0
    n_blk8 = n_sh + n_g4
    PW_BLK, RES_BLK = 0, 1          # bf16 block indices

    xp8_d = nc.dram_tensor("xp8", [C, HP * WP], f8, kind="ExternalInput")
    wt8_d = nc.dram_tensor("wt8", [C, n_blk8 * C], f8, kind="ExternalInput")
    wtb_d = nc.dram_tensor("wtb", [C, 2 * C], bf16, kind="ExternalInput")
    xrb_d = nc.dram_tensor("xrb", [C, H * W], bf16, kind="ExternalInput")
    ks_d = nc.dram_tensor("ks", [C, len(g4_taps)], f32, kind="ExternalInput")
    out_d = nc.dram_tensor("out", [C, H * W], f32, kind="ExternalOutput")

    with tile.TileContext(nc) as tc:
        with (
            tc.tile_pool(name="const", bufs=1) as const,
            tc.tile_pool(name="outp", bufs=3) as outp,
            tc.tile_pool(name="ypA", bufs=1) as ypA,
            tc.tile_pool(name="ypB", bufs=1) as ypB,
            tc.tile_pool(name="ypC", bufs=3) as ypC,
            tc.tile_pool(name="psum", bufs=4, space="PSUM") as psum_pool,
        ):
            xp8_sb = const.tile([C, HP * WP], f8)
            wt8_sb = const.tile([C, n_blk8 * C], f8)
            wtb_sb = const.tile([C, 2 * C], bf16)
            xrb_sb = const.tile([C, H * W], bf16)
            ks_sb = const.tile([C, len(g4_taps)], f32)

            # SWDGE (nc.gpsimd) fans transfers across the SDMA engines;
            # ordered so SB0 (weights, first rows, xrb quarter 0) starts as
            # early as possible.
            nc.sync.dma_start(out=ks_sb, in_=ks_d[:, :])
            ROWS0 = 36
            nc.gpsimd.dma_start(out=xp8_sb[:, :ROWS0 * WP],
                                in_=xp8_d[:, :ROWS0 * WP])
            WT_C0 = 20 * C
            nc.gpsimd.dma_start(out=wt8_sb[:, :WT_C0], in_=wt8_d[:, :WT_C0])
            nc.gpsimd.dma_start(out=wtb_sb, in_=wtb_d[:, :])
            nc.gpsimd.dma_start(out=wt8_sb[:, WT_C0:], in_=wt8_d[:, WT_C0:])
            Q = H * W // 4
            nc.gpsimd.dma_start(out=xrb_sb[:, :Q], in_=xrb_d[:, :Q])
            XP_CHUNK_ROWS = 48
            for r0_ in range(ROWS0, HP, XP_CHUNK_ROWS):
                r1_ = min(r0_ + XP_CHUNK_ROWS, HP)
                nc.gpsimd.dma_start(out=xp8_sb[:, r0_ * WP:r1_ * WP],
                                    in_=xp8_d[:, r0_ * WP:r1_ * WP])
            for q0 in range(Q, H * W, Q):
                nc.gpsimd.dma_start(out=xrb_sb[:, q0:q0 + Q],
                                    in_=xrb_d[:, q0:q0 + Q])

            xp3 = xp8_sb.rearrange("p (r w) -> p r w", w=WP)
            wt83 = wt8_sb.rearrange("p (t c) -> p t c", c=C)
            wtb3 = wtb_sb.rearrange("p (t c) -> p t c", c=C)

            y_pools = {0: ypA, 1: ypB, 2: ypC, 3: ypC, 4: ypC}
            y_sched = {at: (idx, rows) for idx, (at, rows) in enumerate(Y_SCHED)}

            def emit_y(idx, rows, n_taps):
                r_lo, r_hi = rows
                nr = r_hi - r_lo
                pool_key = "C" if idx >= 2 else str(idx)
                y = y_pools[idx].tile([C, nr * W], bf16, name=f"y4_{idx}",
                                      tag=f"y4_{pool_key}")
                for t in range(n_taps):
                    dy, dx = g4_taps[t]
                    xs = xp3[:, PAD + r_lo + dy: PAD + r_lo + dy + nr,
                             PAD + dx: PAD + dx + W]
                    kcol = ks_sb[:, t: t + 1]
                    if t == 0:
                        nc.vector.tensor_scalar_mul(y, xs, kcol)
                    else:
                        nc.vector.scalar_tensor_tensor(
                            y, xs, kcol, y,
                            mybir.AluOpType.mult, mybir.AluOpType.add)
                return (y.rearrange("p (r w) -> p r w", w=W), r_lo)

            def pair_rhs(a0, o1, o2):
                """4-dim AP: two shifted [SUB_ROWS x W] views of xp8, pair
                stride = flat offset delta (DoubleRow moving tensor)."""
                di1, dj1 = o1
                base = xp3[:, a0 + di1: a0 + di1 + SUB_ROWS,
                           PAD + dj1: PAD + dj1 + W]
                delta = (o2[0] - di1) * WP + (o2[1] - dj1)
                dims = [list(d) for d in base.ap]
                nd = [dims[0], [delta, 2]] + dims[1:]
                return AP(base.tensor, base.offset, nd)

            # ---- main loop -------------------------------------------------
            n_sub = SB_ROWS // SUB_ROWS
            cur_y = None
            for s in range(N_SB):
                r0 = s * SB_ROWS
                early = s < RAMP_SB
                if s in y_sched:
                    idx, rows = y_sched[s]
                    n_taps = EARLY_OFF if rows[0] < RAMP_SB * SB_ROWS \
                        else LATE_OFF
                    cur_y = emit_y(idx, rows, n_taps)
                y3, y_base = cur_y
                off_r = r0 - y_base

                # pair list for this region: 34 shared (+5 g4-early)
                pairs = [(wt83[:, 2 * p: 2 * p + 2, :],
                          sh_offsets[2 * p], sh_offsets[2 * p + 1])
                         for p in range(n_sh // 2)]
                if early:
                    pairs += [(wt83[:, n_sh + 2 * p: n_sh + 2 * p + 2, :],
                               g4_offsets[2 * p], g4_offsets[2 * p + 1])
                              for p in range(n_g4 // 2)]

                psum = psum_pool.tile([C, SB_ROWS * W], f32, tag="acc")
                for p, (lhsT, o1, o2) in enumerate(pairs):
                    for u in range(n_sub):
                        a0 = PAD + r0 + u * SUB_ROWS
                        nc.tensor.matmul(
                            psum[:, u * SUB_ROWS * W:(u + 1) * SUB_ROWS * W],
                            lhsT, pair_rhs(a0, o1, o2),
                            start=(p == 0), stop=False,
                            perf_mode=mybir.MatmulPerfMode.DoubleRow)
                for u in range(n_sub):
                    lo = off_r + u * SUB_ROWS
                    nc.tensor.matmul(
                        psum[:, u * SUB_ROWS * W:(u + 1) * SUB_ROWS * W],
                        wtb3[:, PW_BLK, :],
                        y3[:, lo: lo + SUB_ROWS, :],
                        start=False, stop=False)
                # residual + bias + boundary corrections, pre-combined in xrb
                for u in range(n_sub):
                    q0 = (r0 + u * SUB_ROWS) * W
                    nc.tensor.matmul(
                        psum[:, u * SUB_ROWS * W:(u + 1) * SUB_ROWS * W],
                        wtb3[:, RES_BLK, :],
                        xrb_sb[:, q0: q0 + SUB_ROWS * W],
                        start=False, stop=(u == n_sub - 1))

                # idle Scalar engine evacuates: out = psum * 2^-7
                out_sb = outp.tile([C, SB_ROWS * W], f32)
                nc.scalar.activation(out_sb, psum,
                                     mybir.ActivationFunctionType.Copy,
                                     bias=0.0, scale=1.0 / SCALE)
                nc.gpsimd.dma_start(out=out_d[:, r0 * W:(r0 + SB_ROWS) * W],
                                    in_=out_sb)
    nc.finalize()
    return nc


def _make_in_maps(inputs):
    x = np.ascontiguousarray(inputs["x"], dtype=np.float32)
    wt8, wtb, ks, beff, strips, cmats, meta = _build_weights(inputs)
    if "nc" not in _CACHE:
        _CACHE["nc"] = _build_program(meta)

    import ml_dtypes
    bf = ml_dtypes.bfloat16
    f8 = ml_dtypes.float8_e4m3
    xpad = np.zeros((B, C, HP, WP), f8)
    xpad[:, :, PAD:PAD + H, PAD:PAD + W] = x.astype(f8)
    corr = _host_corrections(x, strips, cmats)
    xrb = (x + beff[None, :, None, None] + corr).astype(bf)
    wt8_q = wt8.astype(f8)
    wtb_q = wtb.astype(bf)
    ksc = np.ascontiguousarray(ks)
    return [
        {
            "xp8": np.ascontiguousarray(xpad[b].reshape(C, HP * WP)),
            "wt8": wt8_q,
            "wtb": wtb_q,
            "xrb": np.ascontiguousarray(xrb[b].reshape(C, H * W)),
            "ks": ksc,
        }
        for b in range(B)
    ]


def kernel(**inputs):
    in_maps = _make_in_maps(inputs)
    from concourse.bass_utils import run_bass_kernel_spmd
    res = run_bass_kernel_spmd(_CACHE["nc"], in_maps, core_ids=list(range(N_CORES)))
    out = np.stack([res.results[b]["out"].reshape(C, H, W) for b in range(B)])
    return out.astype(np.float32)


# revision 27
# speedup vs baseline: 1.0088x; 1.0025x over previous
"""Trainium2 Bass kernel for nn_LongRangeDW (dense_cnn).

The module is entirely linear in x:
  s = nnstacking(x)                        (5 shifted copies, clipped to window)
  y = dw1(s) + dw2(s) + dw3(s)             (depthwise 1x1 + 3x3 d8 + 3x3 d12)
  out = pw(y) + x                          (pointwise 5C->C + residual)

Folding the depthwise taps into the pointwise gives, per nnstacking group g
with shift sigma_g and tap tau:
  out[o, p] = sum_{g,t} (W4_g diag(k_{g,t}))[o,:] @ xe[:, p + tau_t + sigma_g]
              + beff[o] + x[o, p]
with xe = zero-extended x: 85 distinct offsets. The folded weight matrices are
quantized to fp8e4 (scaled by 2^7 into e4m3's normal range) and applied two
offsets per instruction with MatmulPerfMode.DoubleRow: lhsT [C, 2, C] holds
two 128x128 blocks, the moving tensor is a 4-dim AP pairing two shifted views
of the fp8 zero-padded SBUF image (pair stride = flat offset delta). Measured
on HW this runs ~1.84x bf16 per folded term.

N_OFF taps of the center group are NOT run as matmuls: their depthwise output
y4 is built by per-channel-scalar tap FMAs on the Vector engine (which is
otherwise idle), followed by one bf16 pointwise matmul term. The first y tile
is emitted in a small 8-row chunk so SB0's pointwise matmul isn't blocked
behind a full 32-row tap batch.

Boundary exactness: composing clipped shifts with zero-padded convs is NOT
the padded composite; the mismatch lives on 8 one-pixel strips (output
rows/cols {7,11,116,120}) reading x's 4 border lines. Those corrections are
linear in x and are folded into xrb on the HOST (exact fp32), so the device
sees no strip machinery at all.

PSUM holds 2^7x the true result: fp8 weights carry 2^7, the residual+bias+
correction tensor xrb enters PSUM through a (2^7 * I) bf16 matmul, and the
idle Scalar engine evacuates with a single Copy-with-scale(2^-7) activation.
The Vector engine runs nothing but tap FMAs; psum slots release through ACT.

Data parallel: batch B=8 -> one image per NeuronCore.
"""

import sys

import numpy as np

sys.path.insert(0, "/opt/trn_rl_repo")

B, C, H, W = 8, 128, 128, 128
PAD = 14            # max |offset| = 13, rounded up
HP = H + 2 * PAD
WP = W + 2 * PAD
N_CORES = 8
SB_ROWS = 8         # output rows per super-block (psum tile = 2 banks)
N_SB = H // SB_ROWS
SUB_ROWS = 4        # rows per matmul (out free dim 512 = one PSUM bank)
SCALE = 128.0       # fp8 weight pre-scale (2^7); descaled at evacuation

SHIFTS = [(1, 0), (-1, 0), (0, 1), (0, -1), (0, 0)]  # nnstacking groups

# Taps of the center group (g=4) are offloaded to the Vector engine as
# per-channel tap FMAs. The offload count ramps across the image: the DVE's
# cumulative y-tile supply curve must stay ahead of the PE's consumption
# curve, and the DVE starts from zero, so early super-blocks offload fewer
# taps (more matmul terms) and late super-blocks offload the whole group.
EARLY_OFF = 7        # g4 taps offloaded for SB 0..RAMP_SB-1
LATE_OFF = 17        # g4 taps offloaded for SB RAMP_SB..15 (all of them)
RAMP_SB = 8


# --------------------------------------------------------------------------
# host-side operator folding
# --------------------------------------------------------------------------

def _group_taps(w1, w2, w3, g):
    """All 17 taps of group g as {(di, dj): kvec[C]} (shift folded in)."""
    sy, sx = SHIFTS[g]
    sl = slice(g * C, (g + 1) * C)
    taps = {}

    def add(di, dj, kv):
        v = taps.setdefault((di, dj), np.zeros(C, np.float64))
        v += kv.astype(np.float64)

    add(sy, sx, w1[sl, 0, 0, 0])
    for w, d in ((w2, 8), (w3, 12)):
        for a in range(3):
            for b in range(3):
                add(sy + (a - 1) * d, sx + (b - 1) * d, w[sl, 0, a, b])
    return taps


def _build_terms(w1, w2, w3, w4):
    """Folds taps into matmul terms, split into the 68 'shared' terms (groups
    0-3, both regions) and the 10 'g4-extra' terms (center-group taps kept as
    matmuls only in the early region).

    Returns dict with:
      sh_offsets/sh_mats: 68 shared terms (paired consecutively)
      g4_offsets/g4_mats: g4 taps[EARLY_OFF:] as matmul terms (early region)
      g4_taps: all 17 sorted g4 tap offsets,  g4_kmat [C, 17]
      w4g4 [C, C]: pointwise block of group 4
    """
    w4m = w4[:, :, 0, 0].astype(np.float64)  # [C, 5C]
    mat_terms = {}
    for g in range(4):
        taps = _group_taps(w1, w2, w3, g)
        for o in sorted(taps):
            M = mat_terms.setdefault(o, np.zeros((C, C), np.float64))
            M += w4m[:, g * C:(g + 1) * C] * taps[o][None, :]
    sh_offsets = sorted(mat_terms)
    sh_mats = np.stack([mat_terms[o] for o in sh_offsets]).astype(np.float32)

    g4 = _group_taps(w1, w2, w3, 4)
    g4_taps = sorted(g4)
    g4_kmat = np.stack([g4[o] for o in g4_taps], axis=1).astype(np.float32)
    w4g4 = w4m[:, 4 * C:5 * C]
    g4_offsets = g4_taps[EARLY_OFF:]
    g4_mats = np.stack([w4g4 * g4[o][None, :] for o in g4_offsets]
                       ).astype(np.float32)
    return dict(sh_offsets=sh_offsets, sh_mats=sh_mats,
                g4_offsets=g4_offsets, g4_mats=g4_mats,
                g4_taps=g4_taps, g4_kmat=g4_kmat,
                w4g4=w4g4.astype(np.float32))


def _build_corrections(w2, w3, w4):
    """24 strip-correction terms (matrices already NEGATED for accumulation).

    Strips j<4: column strips (out col px, read x col src, row shift ty);
    j>=4: row strips. Each strip has 3 taps."""
    w4m = w4[:, :, 0, 0].astype(np.float64)
    strips, mats = [], []
    specs = [
        ("col", 2, 8), ("col", 2, 12), ("col", 3, 12), ("col", 3, 8),
        ("row", 0, 8), ("row", 0, 12), ("row", 1, 12), ("row", 1, 8),
    ]
    for kind, g, d in specs:
        sy, sx = SHIFTS[g]
        sl = slice(g * C, (g + 1) * C)
        w = w2 if d == 8 else w3
        if kind == "col":
            border = -1 if sx == 1 else W
            fixed_out = border - (-d if sx == 1 else d)
            src = border + sx
            shifts = [-d, 0, d]                     # ty values
            tap_b = 0 if sx == 1 else 2
            kvs = [w[sl, 0, a, tap_b] for a in range(3)]
        else:
            border = -1 if sy == 1 else H
            fixed_out = border - (-d if sy == 1 else d)
            src = border + sy
            shifts = [-d, 0, d]                     # tx values
            tap_a = 0 if sy == 1 else 2
            kvs = [w[sl, 0, tap_a, b] for b in range(3)]
        strips.append(dict(kind=kind, fixed_out=fixed_out, src=src, shifts=shifts))
        for kv in kvs:
            mats.append(-(w4m[:, sl] * kv.astype(np.float64)[None, :]))
    return strips, np.stack(mats).astype(np.float32)


def _host_corrections(x, strips, cmats):
    """Per-image boundary corrections, exact fp32: returns [B, C, H, W]."""
    corr = np.zeros((B, C, H, W), np.float32)
    for j, st in enumerate(strips):
        for i, sh in enumerate(st["shifts"]):
            cm = cmats[3 * j + i]                   # [C(out), C(in)], negated
            a, b2 = max(0, -sh), min(H, H - sh)
            if st["kind"] == "col":
                src = x[:, :, a + sh:b2 + sh, st["src"]]        # [B, C, n]
                add = np.einsum("oc,bcn->bon", cm, src)
                corr[:, :, a:b2, st["fixed_out"]] += add
            else:
                src = x[:, :, st["src"], a + sh:b2 + sh]
                add = np.einsum("oc,bcn->bon", cm, src)
                corr[:, :, st["fixed_out"], a:b2] += add
    return corr


def _build_weights(inputs):
    w1, w2, w3, w4 = inputs["w1"], inputs["w2"], inputs["w3"], inputs["w4"]
    b1, b2, b3, b4 = inputs["b1"], inputs["b2"], inputs["b3"], inputs["b4"]
    terms = _build_terms(w1, w2, w3, w4)
    strips, cmats = _build_corrections(w2, w3, w4)

    # fp8 tensor: 68 shared blocks (34 pairs) then 10 g4-early blocks
    # (5 pairs), pair members adjacent in the block dim. Scaled by SCALE.
    mats = np.concatenate([terms["sh_mats"], terms["g4_mats"]], axis=0)
    wt8 = np.ascontiguousarray(
        (mats * SCALE).transpose(2, 0, 1).reshape(C, -1)).astype(np.float32)

    # bf16 tensor: g4 pointwise block (scaled) + SCALE*I for the
    # residual/bias/correction injection.
    bf_blocks = np.stack([terms["w4g4"] * SCALE,
                          np.eye(C, dtype=np.float32) * SCALE])
    wtb = np.ascontiguousarray(
        bf_blocks.transpose(2, 0, 1).reshape(C, -1)).astype(np.float32)

    ks = np.ascontiguousarray(terms["g4_kmat"])  # [C, 17]
    w4m = w4[:, :, 0, 0].astype(np.float64)
    beff = (b4.astype(np.float64)
            + w4m @ (b1 + b2 + b3).astype(np.float64)).astype(np.float32)
    meta = dict(sh_offsets=terms["sh_offsets"],
                g4_offsets=terms["g4_offsets"],
                g4_taps=terms["g4_taps"])
    return wt8, wtb, ks, beff, strips, cmats, meta


# --------------------------------------------------------------------------
# device program
# --------------------------------------------------------------------------

_CACHE = {}

# y-tile schedule: (emit_at_sb, rows_covered); first tile split 8+24 so SB0's
# pointwise matmul is unblocked after a short tap batch.
Y_SCHED = [(0, (0, 8)), (1, (8, 32)), (4, (32, 64)), (8, (64, 96)),
           (12, (96, 128))]


def _build_program(meta):
    import concourse.bacc as bacc
    import concourse.mybir as mybir
    import concourse.tile as tile
    from concourse.ap import AP

    nc = bacc.Bacc("TRN2", target_bir_lowering=False)
    f32 = mybir.dt.float32
    bf16 = mybir.dt.bfloat16
    f8 = mybir.dt.float8e4

    sh_offsets = meta["sh_offsets"]
    g4_offsets = meta["g4_offsets"]
    g4_taps = meta["g4_taps"]
    n_sh = len(sh_offsets)          # 68 shared terms -> 34 pairs
    n_g4 = len(g4_offsets)          # 10 early-only terms -> 5 pairs
    assert n_sh % 2 == 0 and n_g4 % 2 == 0
    n_blk8 = n_sh + n_g4
    PW_BLK, RES_BLK = 0, 1          # bf16 block indices

    xp8_d = nc.dram_tensor("xp8", [C, HP * WP], f8, kind="ExternalInput")
    wt8_d = nc.dram_tensor("wt8", [C, n_blk8 * C], f8, kind="ExternalInput")
    wtb_d = nc.dram_tensor("wtb", [C, 2 * C], bf16, kind="ExternalInput")
    xrb_d = nc.dram_tensor("xrb", [C, H * W], bf16, kind="ExternalInput")
    ks_d = nc.dram_tensor("ks", [C, len(g4_taps)], f32, kind="ExternalInput")
    out_d = nc.dram_tensor("out", [C, H * W], f32, kind="ExternalOutput")

    with tile.TileContext(nc) as tc:
        with (
            tc.tile_pool(name="const", bufs=1) as const,
            tc.tile_pool(name="outp", bufs=3) as outp,
            tc.tile_pool(name="ypA", bufs=1) as ypA,
            tc.tile_pool(name="ypB", bufs=1) as ypB,
            tc.tile_pool(name="ypC", bufs=3) as ypC,
            tc.tile_pool(name="psum", bufs=4, space="PSUM") as psum_pool,
        ):
            xp8_sb = const.tile([C, HP * WP], f8)
            wt8_sb = const.tile([C, n_blk8 * C], f8)
            wtb_sb = const.tile([C, 2 * C], bf16)
            xrb_sb = const.tile([C, H * W], bf16)
            ks_sb = const.tile([C, len(g4_taps)], f32)

            # SWDGE (nc.gpsimd) fans transfers across the SDMA engines;
            # ordered so SB0 (weights, first rows, xrb quarter 0) starts as
            # early as possible.
            nc.sync.dma_start(out=ks_sb, in_=ks_d[:, :])
            ROWS0 = 36
            nc.gpsimd.dma_start(out=xp8_sb[:, :ROWS0 * WP],
                                in_=xp8_d[:, :ROWS0 * WP])
            WT_C0 = 20 * C
            nc.gpsimd.dma_start(out=wt8_sb[:, :WT_C0], in_=wt8_d[:, :WT_C0])
            nc.gpsimd.dma_start(out=wtb_sb, in_=wtb_d[:, :])
            nc.gpsimd.dma_start(out=wt8_sb[:, WT_C0:], in_=wt8_d[:, WT_C0:])
            Q = H * W // 4
            nc.gpsimd.dma_start(out=xrb_sb[:, :Q], in_=xrb_d[:, :Q])
            XP_CHUNK_ROWS = 48
            for r0_ in range(ROWS0, HP, XP_CHUNK_ROWS):
                r1_ = min(r0_ + XP_CHUNK_ROWS, HP)
                nc.gpsimd.dma_start(out=xp8_sb[:, r0_ * WP:r1_ * WP],
                                    in_=xp8_d[:, r0_ * WP:r1_ * WP])
            for q0 in range(Q, H * W, Q):
                nc.gpsimd.dma_start(out=xrb_sb[:, q0:q0 + Q],
                                    in_=xrb_d[:, q0:q0 + Q])

            xp3 = xp8_sb.rearrange("p (r w) -> p r w", w=WP)
            wt83 = wt8_sb.rearrange("p (t c) -> p t c", c=C)
            wtb3 = wtb_sb.rearrange("p (t c) -> p t c", c=C)

            y_pools = {0: ypA, 1: ypB, 2: ypC, 3: ypC, 4: ypC}
            y_sched = {at: (idx, rows) for idx, (at, rows) in enumerate(Y_SCHED)}

            def emit_y(idx, rows, n_taps):
                r_lo, r_hi = rows
                nr = r_hi - r_lo
                pool_key = "C" if idx >= 2 else str(idx)
                y = y_pools[idx].tile([C, nr * W], bf16, name=f"y4_{idx}",
                                      tag=f"y4_{pool_key}")
                for t in range(n_taps):
                    dy, dx = g4_taps[t]
                    xs = xp3[:, PAD + r_lo + dy: PAD + r_lo + dy + nr,
                             PAD + dx: PAD + dx + W]
                    kcol = ks_sb[:, t: t + 1]
                    if t == 0:
                        nc.vector.tensor_scalar_mul(y, xs, kcol)
                    else:
                        nc.vector.scalar_tensor_tensor(
                            y, xs, kcol, y,
                            mybir.AluOpType.mult, mybir.AluOpType.add)
                return (y.rearrange("p (r w) -> p r w", w=W), r_lo)

            def pair_rhs(a0, o1, o2):
                """4-dim AP: two shifted [SUB_ROWS x W] views of xp8, pair
                stride = flat offset delta (DoubleRow moving tensor)."""
                di1, dj1 = o1
                base = xp3[:, a0 + di1: a0 + di1 + SUB_ROWS,
                           PAD + dj1: PAD + dj1 + W]
                delta = (o2[0] - di1) * WP + (o2[1] - dj1)
                dims = [list(d) for d in base.ap]
                nd = [dims[0], [delta, 2]] + dims[1:]
                return AP(base.tensor, base.offset, nd)

            # ---- main loop -------------------------------------------------
            n_sub = SB_ROWS // SUB_ROWS
            cur_y = None
            for s in range(N_SB):
                r0 = s * SB_ROWS
                early = s < RAMP_SB
                if s in y_sched:
                    idx, rows = y_sched[s]
                    n_taps = EARLY_OFF if rows[0] < RAMP_SB * SB_ROWS \
                        else LATE_OFF
                    cur_y = emit_y(idx, rows, n_taps)
                y3, y_base = cur_y
                off_r = r0 - y_base

                # pair list for this region: 34 shared (+5 g4-early)
                pairs = [(wt83[:, 2 * p: 2 * p + 2, :],
                          sh_offsets[2 * p], sh_offsets[2 * p + 1])
                         for p in range(n_sh // 2)]
                if early:
                    pairs += [(wt83[:, n_sh + 2 * p: n_sh + 2 * p + 2, :],
                               g4_offsets[2 * p], g4_offsets[2 * p + 1])
                              for p in range(n_g4 // 2)]

                psum = psum_pool.tile([C, SB_ROWS * W], f32, tag="acc")
                for p, (lhsT, o1, o2) in enumerate(pairs):
                    for u in range(n_sub):
                        a0 = PAD + r0 + u * SUB_ROWS
                        nc.tensor.matmul(
                            psum[:, u * SUB_ROWS * W:(u + 1) * SUB_ROWS * W],
                            lhsT, pair_rhs(a0, o1, o2),
                            start=(p == 0), stop=False,
                            perf_mode=mybir.MatmulPerfMode.DoubleRow)
                for u in range(n_sub):
                    lo = off_r + u * SUB_ROWS
                    nc.tensor.matmul(
                        psum[:, u * SUB_ROWS * W:(u + 1) * SUB_ROWS * W],
                        wtb3[:, PW_BLK, :],
                        y3[:, lo: lo + SUB_ROWS, :],
                        start=False, stop=False)
                # residual + bias + boundary corrections, pre-combined in xrb
                for u in range(n_sub):
                    q0 = (r0 + u * SUB_ROWS) * W
                    nc.tensor.matmul(
                        psum[:, u * SUB_ROWS * W:(u + 1) * SUB_ROWS * W],
                        wtb3[:, RES_BLK, :],
                        xrb_sb[:, q0: q0 + SUB_ROWS * W],
                        start=False, stop=(u == n_sub - 1))

                # idle Scalar engine evacuates: out = psum * 2^-7
                out_sb = outp.tile([C, SB_ROWS * W], f32)
                nc.scalar.activation(out_sb, psum,
                                     mybir.ActivationFunctionType.Copy,
                                     bias=0.0, scale=1.0 / SCALE)
                nc.gpsimd.dma_start(out=out_d[:, r0 * W:(r0 + SB_ROWS) * W],
                                    in_=out_sb)
    nc.finalize()
    return nc


def _make_in_maps(inputs):
    x = np.ascontiguousarray(inputs["x"], dtype=np.float32)
    wt8, wtb, ks, beff, strips, cmats, meta = _build_weights(inputs)
    if "nc" not in _CACHE:
        _CACHE["nc"] = _build_program(meta)

    import ml_dtypes
    bf = ml_dtypes.bfloat16
    f8 = ml_dtypes.float8_e4m3
    xpad = np.zeros((B, C, HP, WP), f8)
    xpad[:, :, PAD:PAD + H, PAD:PAD + W] = x.astype(f8)
    corr = _host_corrections(x, strips, cmats)
    xrb = (x + beff[None, :, None, None] + corr).astype(bf)
    wt8_q = wt8.astype(f8)
    wtb_q = wtb.astype(bf)
    ksc = np.ascontiguousarray(ks)
    return [
        {
            "xp8": np.ascontiguousarray(xpad[b].reshape(C, HP * WP)),
            "wt8": wt8_q,
            "wtb": wtb_q,
            "xrb": np.ascontiguousarray(xrb[b].reshape(C, H * W)),
            "ks": ksc,
        }
        for b in range(B)
    ]


def kernel(**inputs):
    in_maps = _make_in_maps(inputs)
    from concourse.bass_utils import run_bass_kernel_spmd
    res = run_bass_kernel_spmd(_CACHE["nc"], in_maps, core_ids=list(range(N_CORES)))
    out = np.stack([res.results[b]["out"].reshape(C, H, W) for b in range(B)])
    return out.astype(np.float32)
